# revision 4
# baseline (speedup 1.0000x reference)
"""Trainium2 Bass kernel for nn_ChempropBlock (GNN message passing).

Strategy (8 NeuronCores, SPMD single NEFF):
  - Edges sharded by dest-node range (core c owns nodes [c*VS,(c+1)*VS) and all
    edges pointing to them) so segment_sum is core-local.
  - Within a core, edges grouped by the owning shard of their rev_index target
    so the rev-gather uses shard-local int16 indices (dma_gather limit).
  - Per layer (chunk loop over edges):
      em' = relu(eh_table[rev]) - NM_table[src]     (transpose-mode dma_gather
            gives feature-major stripes directly; relu+sub fused on DVE)
      eh_new = em' @ (-W)^T + b + eh_old            (PE: 4 stripe matmuls with
            gathered data as stationary; bias via K=1 matmul; residual via
            identity matmul; accumulated in fp32 PSUM)
  - Segment-sum as one-hot matmul (collision-free, unlike dma_scatter_add whose
    concurrent RMW loses colliding updates): a separate pass re-gathers eh rows
    in dest-sorted order and multiplies with precomputed one-hot A tiles,
    accumulating each 128-node window in PSUM.
  - NM shard and eh shard AllGather'd between layers (bf16 tables).
  - Final: node_hiddens = one-hot scatter of eh_3 (fp32, no relu).
"""

import sys

if "/opt/trn_rl_repo" not in sys.path:
    sys.path.insert(0, "/opt/trn_rl_repo")

import numpy as np
import ml_dtypes

import concourse.bacc as bacc
import concourse.mybir as mybir
import concourse.tile as tile
from concourse.bass_utils import run_bass_kernel_spmd

NCORES = 8
CHUNK = 512
F32 = mybir.dt.float32
BF16 = mybir.dt.bfloat16
I16 = mybir.dt.int16

_BUILD_CACHE = {}


def _wrap16(a):
    """[N] -> [128, N/16] int16: 16-partition wrap replicated for 8 Q7 cores."""
    n = a.shape[0]
    assert n % 16 == 0
    return np.tile(a.astype(np.int16).reshape(n // 16, 16).T, (8, 1)).copy()


def _ceil_to(x, m):
    return -(-x // m) * m


def _build_nc(V, E_pad, D, DEPTH, VSP, chunk_groups, win_tiles):
    """Build the SPMD NEFF.

    chunk_groups[i] = rev-owner group of update chunk i.
    win_tiles[w] = number of 128-edge scatter tiles for node window w.
    """
    nc = bacc.Bacc("TRN2", target_bir_lowering=False, debug=False,
                   num_devices=NCORES)
    NST = D // 128
    nchunks = E_pad // CHUNK
    NWIN = len(win_tiles)
    SCT = int(sum(win_tiles))          # total scatter tiles
    ESC = SCT * 128                    # scatter-pass slots

    nf = nc.dram_tensor("nf", [V, D], F32, kind="ExternalInput")
    ef = nc.dram_tensor("ef", [E_pad, D], F32, kind="ExternalInput")
    wqneg = nc.dram_tensor("wqneg", [DEPTH * NST * 128, D], BF16,
                           kind="ExternalInput")
    bias = nc.dram_tensor("bias", [1, DEPTH * D], F32, kind="ExternalInput")
    ident = nc.dram_tensor("ident", [128, 128], BF16, kind="ExternalInput")
    ones = nc.dram_tensor("ones", [1, 128], F32, kind="ExternalInput")
    g1_idx = nc.dram_tensor("g1_idx", [128, E_pad // 16], I16, kind="ExternalInput")
    rev_idx = nc.dram_tensor("rev_idx", [128, E_pad // 16], I16, kind="ExternalInput")
    srcraw_idx = nc.dram_tensor("srcraw_idx", [128, E_pad // 16], I16,
                                kind="ExternalInput")
    sc_idx = nc.dram_tensor("sc_idx", [128, ESC // 16], I16, kind="ExternalInput")
    a_bf = nc.dram_tensor("a_bf", [SCT * 128, 128], BF16, kind="ExternalInput")
    a_f32 = nc.dram_tensor("a_f32", [SCT * 128, 128], F32, kind="ExternalInput")

    nh_out = nc.dram_tensor("nh_out", [VSP, D], F32, kind="ExternalOutput")
    eh_out = nc.dram_tensor("eh_out", [E_pad, D], F32, kind="ExternalOutput")

    eh_shard = [nc.dram_tensor(f"eh_shard{i}", [E_pad, D], BF16, kind="Internal")
                for i in range(2)]
    eh_table = [nc.dram_tensor(f"eh_table{i}", [NCORES * E_pad, D], BF16,
                               kind="Internal", addr_space="Shared")
                for i in range(2)]
    nm_bf = nc.dram_tensor("nm_bf", [VSP, D], BF16, kind="Internal")
    nm_table = [nc.dram_tensor(f"nm_table{i}", [NCORES * VSP, D], BF16,
                               kind="Internal", addr_space="Shared")
                for i in range(2)]

    rg = [list(range(NCORES))]

    def rows(t, s, n):
        return t[s:s + n, :].rearrange("(a p) d -> p a d", p=128)

    with tile.TileContext(nc) as tc:
        with (
            tc.tile_pool(name="cst", bufs=1) as cst,
            tc.tile_pool(name="sb", bufs=2) as sb,
            tc.tile_pool(name="ps", bufs=1, space="PSUM") as pst,
            tc.tile_pool(name="psn", bufs=2, space="PSUM") as psn,
        ):
            # resident constants
            g1_t = cst.tile([128, E_pad // 16], I16)
            nc.sync.dma_start(g1_t[:], g1_idx[:])
            rev_t = cst.tile([128, E_pad // 16], I16)
            nc.sync.dma_start(rev_t[:], rev_idx[:])
            srcr_t = cst.tile([128, E_pad // 16], I16)
            nc.sync.dma_start(srcr_t[:], srcraw_idx[:])
            sc_t = cst.tile([128, ESC // 16], I16)
            nc.sync.dma_start(sc_t[:], sc_idx[:])
            a_t = cst.tile([128, SCT, 128], BF16)
            nc.sync.dma_start(a_t[:], a_bf[:].rearrange("(a p) v -> p a v", p=128))
            wq_t = cst.tile([128, DEPTH * NST, D], BF16)
            nc.sync.dma_start(
                wq_t[:], wqneg[:].rearrange("(a p) d -> p a d", p=128))
            bias_t = cst.tile([1, DEPTH * D], F32)
            nc.sync.dma_start(bias_t[:], bias[:])
            id_t = cst.tile([128, 128], BF16)
            nc.sync.dma_start(id_t[:], ident[:])
            ones_t = cst.tile([1, 128], F32)
            nc.sync.dma_start(ones_t[:], ones[:])

            def idx_slice(t, s, n=CHUNK):
                return t[:, s // 16:(s + n) // 16]

            def scatter_pass(src_tensor, l_or_none):
                """NM_w = sum_t A_t^T @ relu(src[sigma]) per 128-node window.
                l_or_none = layer index for NM (bf16+AG) or None for final
                node_hiddens (f32, no relu, fp32 A)."""
                final = l_or_none is None
                MAXG = 4
                gt = 0
                for w in range(NWIN):
                    tw = win_tiles[w]
                    ps_nm = psn.tile([128, D], F32, space="PSUM", tag="psnm")
                    for t0 in range(0, tw, MAXG):
                        nt = min(MAXG, tw - t0)
                        s = (gt + t0) * 128
                        if final:
                            rowsw = sb.tile([128, MAXG, D], F32, tag="nfg")
                            nc.gpsimd.dma_gather(
                                rowsw[:, :nt, :], src_tensor[:],
                                idx_slice(sc_t, s, nt * 128),
                                num_idxs=nt * 128, num_idxs_reg=nt * 128,
                                elem_size=D)
                            aw = sb.tile([128, MAXG, 128], F32, tag="ef")
                            nc.sync.dma_start(
                                aw[:, :nt, :],
                                a_f32[s:s + nt * 128, :]
                                .rearrange("(a p) v -> p a v", p=128))
                            rhs_w = rowsw
                        else:
                            rowsw = sb.tile([128, MAXG, D], BF16, tag="g1")
                            nc.gpsimd.dma_gather(
                                rowsw[:, :nt, :], src_tensor[:],
                                idx_slice(sc_t, s, nt * 128),
                                num_idxs=nt * 128, num_idxs_reg=nt * 128,
                                elem_size=D)
                            relu_w = sb.tile([128, MAXG, D], BF16, tag="g2")
                            nc.vector.tensor_scalar_max(
                                relu_w[:, :nt, :], rowsw[:, :nt, :], 0.0)
                            rhs_w = relu_w
                        for t in range(nt):
                            lhsT = (aw[:, t, :] if final
                                    else a_t[:, gt + t0 + t, :])
                            nc.tensor.matmul(ps_nm[:], lhsT=lhsT,
                                             rhs=rhs_w[:, t, :],
                                             start=(t0 + t == 0),
                                             stop=(t0 + t == tw - 1))
                    nmw = sb.tile([128, D], F32 if final else BF16, tag="nmw")
                    nc.scalar.activation(nmw[:], ps_nm[:],
                                         mybir.ActivationFunctionType.Copy)
                    if final:
                        nc.sync.dma_start(nh_out[w * 128:(w + 1) * 128, :],
                                          nmw[:])
                    else:
                        nc.sync.dma_start(nm_bf[w * 128:(w + 1) * 128, :],
                                          nmw[:])
                    gt += tw
                if not final:
                    nc.gpsimd.collective_compute(
                        "AllGather", mybir.AluOpType.bypass, replica_groups=rg,
                        ins=[nm_bf[:].opt()],
                        outs=[nm_table[l_or_none][:].opt()])

            def eh_ag(l):
                nc.gpsimd.collective_compute(
                    "AllGather", mybir.AluOpType.bypass, replica_groups=rg,
                    ins=[eh_shard[l][:].opt()], outs=[eh_table[l][:].opt()])

            # ---------------- INIT: eh_0 = nf[src] + ef ---------------------
            for ci in range(nchunks):
                s = ci * CHUNK
                nf_g = sb.tile([128, CHUNK // 128, D], F32, tag="nfg")
                nc.gpsimd.dma_gather(
                    nf_g[:], nf[:], idx_slice(srcr_t, s),
                    num_idxs=CHUNK, num_idxs_reg=CHUNK, elem_size=D)
                ef_t = sb.tile([128, CHUNK // 128, D], F32, tag="ef")
                nc.sync.dma_start(ef_t[:], rows(ef, s, CHUNK))
                eh0 = sb.tile([128, CHUNK // 128, D], BF16, tag="ehnew")
                nc.vector.tensor_add(eh0[:], nf_g[:], ef_t[:])
                nc.sync.dma_start(rows(eh_shard[0], s, CHUNK), eh0[:])
            scatter_pass(eh_shard[0], 0)
            eh_ag(0)

            # ---------------- LAYERS ---------------------------------------
            for l in range(DEPTH):
                cur, nxt = l % 2, (l + 1) % 2
                last = l == DEPTH - 1
                for ci in range(nchunks):
                    s = ci * CHUNK
                    g = chunk_groups[ci]
                    g1 = sb.tile([128, NST, CHUNK], BF16, tag="g1")
                    nc.gpsimd.dma_gather(
                        g1[:], nm_table[cur][:], idx_slice(g1_t, s),
                        num_idxs=CHUNK, num_idxs_reg=CHUNK, elem_size=D,
                        transpose=True)
                    g2 = sb.tile([128, NST, CHUNK], BF16, tag="g2")
                    nc.gpsimd.dma_gather(
                        g2[:], eh_table[cur][g * E_pad:(g + 1) * E_pad, :],
                        idx_slice(rev_t, s),
                        num_idxs=CHUNK, num_idxs_reg=CHUNK, elem_size=D,
                        transpose=True)
                    # em' = relu(g2) - g1   (weights are negated)
                    em = sb.tile([128, NST, CHUNK], BF16, tag="em")
                    nc.vector.scalar_tensor_tensor(
                        em[:], g2[:], 0.0, g1[:],
                        op0=mybir.AluOpType.max, op1=mybir.AluOpType.subtract)
                    ehold = sb.tile([128, CHUNK // 128, D], BF16, tag="ehold")
                    nc.sync.dma_start(ehold[:], rows(eh_shard[cur], s, CHUNK))
                    ps = pst.tile([128, CHUNK // 128, D], F32, space="PSUM",
                                  tag="ps")
                    for j in range(CHUNK // 128):
                        pj = ps[:, j, :]
                        nc.tensor.matmul(pj, lhsT=ones_t[:, :],
                                         rhs=bias_t[0:1, l * D:(l + 1) * D],
                                         start=True, stop=False)
                        for q in range(NST):
                            nc.tensor.matmul(
                                pj, lhsT=em[:, q, j * 128:(j + 1) * 128],
                                rhs=wq_t[:, l * NST + q, :],
                                start=False, stop=False)
                        nc.tensor.matmul(pj, lhsT=id_t[:], rhs=ehold[:, j, :],
                                         start=False, stop=True)
                    if not last:
                        ehnew = sb.tile([128, CHUNK // 128, D], BF16,
                                        tag="ehnew")
                        nc.scalar.activation(
                            ehnew[:], ps[:], mybir.ActivationFunctionType.Copy)
                        nc.sync.dma_start(rows(eh_shard[nxt], s, CHUNK),
                                          ehnew[:])
                    else:
                        ehf = sb.tile([128, CHUNK // 128, D], F32, tag="ehf")
                        nc.scalar.activation(
                            ehf[:], ps[:], mybir.ActivationFunctionType.Copy)
                        nc.sync.dma_start(rows(eh_out, s, CHUNK), ehf[:])
                if not last:
                    scatter_pass(eh_shard[nxt], nxt)
                    eh_ag(nxt)
                else:
                    scatter_pass(eh_out, None)

    nc.compile()
    return nc


def _prepare(node_feats, edge_feats, Ws, bs, edge_index, rev_index):
    V, D = node_feats.shape
    E = edge_feats.shape[0]
    DEPTH = Ws.shape[0]
    NST = D // 128
    assert V % NCORES == 0
    VS = V // NCORES
    VSP = _ceil_to(VS, 128)
    NWIN = VSP // 128

    src = np.asarray(edge_index[0]).astype(np.int64)
    dest = np.asarray(edge_index[1]).astype(np.int64)
    rev = np.asarray(rev_index).astype(np.int64)
    owner = dest // VS
    rev_owner = owner[rev]

    # ---- update-pass layout: group by rev-owner, pad groups to CHUNK -------
    cnt = np.zeros((NCORES, NCORES), np.int64)
    np.add.at(cnt, (owner, rev_owner), 1)
    G = np.array([_ceil_to(int(cnt[:, g].max()), CHUNK) for g in range(NCORES)])
    E_pad = int(G.sum())
    bases = np.concatenate([[0], np.cumsum(G)[:-1]])
    assert E_pad - 1 <= np.iinfo(np.int16).max, E_pad

    perm = np.full((NCORES, E_pad), -1, np.int64)
    slot_of_edge = np.empty(E, np.int64)
    for c in range(NCORES):
        mask_c = owner == c
        for g in range(NCORES):
            ids = np.nonzero(mask_c & (rev_owner == g))[0]
            b = bases[g]
            perm[c, b:b + len(ids)] = ids
            slot_of_edge[ids] = b + np.arange(len(ids))
    valid = perm >= 0
    pe = np.where(valid, perm, 0)

    nfp = np.ascontiguousarray(node_feats, dtype=np.float32)
    efp = np.asarray(edge_feats, np.float32)[pe] * valid[:, :, None]
    efp = np.ascontiguousarray(efp, np.float32)

    srcp = src[pe]
    g1 = (srcp // VS) * VSP + (srcp % VS)
    g1[~valid] = 0
    srcraw = np.where(valid, srcp, 0)
    revloc = slot_of_edge[rev[pe]]
    revloc[~valid] = 0

    # ---- scatter-pass layout: dest-sorted windows of 128 local nodes -------
    dloc = dest[pe] - (np.arange(NCORES)[:, None]) * VS   # local dest per slot
    win = dloc // 128                                     # [NC, E_pad]
    wcnt = np.zeros((NCORES, NWIN), np.int64)
    for c in range(NCORES):
        np.add.at(wcnt[c], win[c][valid[c]], 1)
    win_tiles = [max(1, _ceil_to(int(wcnt[:, w].max()), 128) // 128)
                 for w in range(NWIN)]
    SCT = int(sum(win_tiles))
    ESC = SCT * 128
    assert E_pad - 1 <= np.iinfo(np.int16).max

    sc_sigma = np.zeros((NCORES, ESC), np.int64)   # slot in eh order; pad -> 0
    a_host = np.zeros((NCORES, SCT * 128, 128), np.float32)
    wbase = np.concatenate([[0], np.cumsum(win_tiles)[:-1]]) * 128
    for c in range(NCORES):
        vmask = valid[c]
        for w in range(NWIN):
            sel = np.nonzero(vmask & (win[c] == w))[0]   # slots with this window
            b = int(wbase[w])
            n = len(sel)
            sc_sigma[c, b:b + n] = sel
            a_host[c, b + np.arange(n), dloc[c][sel] - w * 128] = 1.0

    wq = (-np.transpose(np.asarray(Ws, np.float32), (0, 2, 1))
          .reshape(DEPTH * NST * 128, D)).astype(ml_dtypes.bfloat16)
    biasp = np.asarray(bs, np.float32).reshape(1, DEPTH * D)
    identp = np.eye(128, dtype=ml_dtypes.bfloat16)
    onesp = np.ones((1, 128), np.float32)

    chunk_groups = []
    for g in range(NCORES):
        chunk_groups += [g] * (int(G[g]) // CHUNK)

    in_maps = []
    for c in range(NCORES):
        in_maps.append(dict(
            nf=nfp, ef=efp[c], wqneg=wq, bias=biasp, ident=identp, ones=onesp,
            g1_idx=_wrap16(g1[c]), rev_idx=_wrap16(revloc[c]),
            srcraw_idx=_wrap16(srcraw[c]), sc_idx=_wrap16(sc_sigma[c]),
            a_bf=a_host[c].astype(ml_dtypes.bfloat16),
            a_f32=a_host[c],
        ))
    meta = dict(V=V, E=E, D=D, DEPTH=DEPTH, VS=VS, VSP=VSP, E_pad=E_pad,
                perm=perm, valid=valid, chunk_groups=tuple(chunk_groups),
                win_tiles=tuple(win_tiles))
    return in_maps, meta


def kernel(node_feats, edge_feats, Ws, bs, edge_index, rev_index):
    in_maps, meta = _prepare(node_feats, edge_feats, Ws, bs,
                             edge_index, rev_index)
    key = (meta["V"], meta["E_pad"], meta["D"], meta["DEPTH"], meta["VSP"],
           meta["chunk_groups"], meta["win_tiles"])
    if key not in _BUILD_CACHE:
        _BUILD_CACHE[key] = _build_nc(meta["V"], meta["E_pad"], meta["D"],
                                      meta["DEPTH"], meta["VSP"],
                                      meta["chunk_groups"], meta["win_tiles"])
    nc = _BUILD_CACHE[key]
    res = run_bass_kernel_spmd(nc, in_maps, core_ids=list(range(NCORES)))

    VS, E, D = meta["VS"], meta["E"], meta["D"]
    nh = np.concatenate([res.results[c]["nh_out"][:VS] for c in range(NCORES)],
                        axis=0)
    eh = np.empty((E, D), np.float32)
    for c in range(NCORES):
        v = meta["valid"][c]
        eh[meta["perm"][c][v]] = res.results[c]["eh_out"][v]
    return nh, eh
